# revision 6
# baseline (speedup 1.0000x reference)
"""LoRA layer kernel for Trainium2, SPMD across 8 NeuronCores.

Computes: out[b,s,h,d] = x[b,s,:] @ W_orig[:,h,d] + SCALE * (x @ A) @ B[:,h,d]

Strategy:
  - LoRA is folded on the host: W_eff = W + SCALE * (A @ B)  (exact
    algebraic identity — standard LoRA weight merge). The device kernel
    is then a pure GEMM out[8192, 2048] = x @ W_eff.
  - Data-parallel over tokens: 8192 tokens -> 1024 per core; W_eff
    replicated (8 MiB bf16 per core).
  - Per core: out[1024, 2048] = x_slice @ W_eff, accumulated over 16
    k-slabs of 128. Loop: chunk-pair (1024 output cols) outer, then
    token-tile PAIRS inner: per k-slab, 4 matmuls of N=512 share the
    256 KiB W slab, so compute (852 ns) outpaces the slab's DMA
    (~715 ns) and the PE stays busy from the first slab's arrival.
  - DMAs are batched (10 input DMAs total) to keep the Tile semaphore
    count low — the NEFF epilogue serially resets every allocated
    semaphore (~115 ns each), so sem count is directly tail latency.
  - All matmul operands bf16, PSUM accumulates fp32, output staged
    bf16 and upcast to fp32 on host.
"""

import numpy as np

# Problem shapes (hardcoded per contract - kernel.py must be self-contained)
B, S, H = 4, 2048, 2048
NH, HD = 16, 128
N = NH * HD            # 2048 output features
RANK = 4
ALPHA = 4.0
SCALE = ALPHA / RANK   # 1.0
NCORES = 8
TOK = B * S            # 8192 tokens total
TPC = TOK // NCORES    # 1024 tokens per core

P = 128                # SBUF partitions
KT = H // P            # 16 contraction slabs
TT = TPC // P          # 8 token tiles per core
CH = 512               # psum bank width (fp32)
NCP = 2                # chunk-pairs (1024 cols each)

_CACHE = {}


def _build_program():
    import concourse.mybir as mybir
    import concourse.tile as tile
    from concourse import bacc

    f32 = mybir.dt.float32
    bf16 = mybir.dt.bfloat16

    nc = bacc.Bacc(None, target_bir_lowering=False, debug=False)

    xt = nc.dram_tensor("xt", [P, TT, KT, P], bf16, kind="ExternalInput")
    w = nc.dram_tensor("w", [NCP, P, KT, 2 * CH], bf16, kind="ExternalInput")
    out = nc.dram_tensor("out", [TPC, N], bf16, kind="ExternalOutput")

    with tile.TileContext(nc) as tc:
        with (
            tc.tile_pool(name="wpool", bufs=1) as wpool,
            tc.tile_pool(name="xpool", bufs=1) as xpool,
            tc.tile_pool(name="opool", bufs=2) as opool,
            tc.tile_pool(name="psum", bufs=1, space="PSUM") as psum,
        ):
            # ---- input DMAs (sync queue, priority order, batched) ----
            # One SBUF tile per chunk-pair for W and one for x; slab-granular
            # DMA only where pacing needs it (first pass), batched after.
            w_sb = [
                wpool.tile([P, KT, 2 * CH], bf16, tag=f"w{cp}",
                           name=f"w_{cp}")
                for cp in range(NCP)
            ]
            x_sb = xpool.tile([P, TT, KT, P], bf16, tag="x", name="x_sb")

            # first pass (t0,t1) is paced by w0 slabs: fine-grained early
            nc.sync.dma_start(w_sb[0][:, 0:1], w[0, :, 0:1])
            nc.sync.dma_start(x_sb[:, 0:2], xt[:, 0:2])
            nc.sync.dma_start(w_sb[0][:, 1:2], w[0, :, 1:2])
            nc.sync.dma_start(w_sb[0][:, 2:4], w[0, :, 2:4])
            nc.sync.dma_start(w_sb[0][:, 4:8], w[0, :, 4:8])
            nc.sync.dma_start(w_sb[0][:, 8:16], w[0, :, 8:16])
            nc.sync.dma_start(x_sb[:, 2:TT], xt[:, 2:TT])
            nc.sync.dma_start(w_sb[1][:, 0:8], w[1, :, 0:8])
            nc.sync.dma_start(w_sb[1][:, 8:16], w[1, :, 8:16])

            # ---- compute: chunk-pair outer, token-tile pairs inner ----
            def close(cp, t, q):
                ot = opool.tile([P, 2 * CH], bf16, tag="o", name=f"o_{cp}_{t}")
                nc.vector.tensor_copy(ot[:], q[:])
                nc.scalar.dma_start(
                    out[t * P:(t + 1) * P, cp * 2 * CH:(cp + 1) * 2 * CH],
                    ot[:])

            def mm(q, t, k, wt, st, sp):
                lhsT = x_sb[:, t, k, :]
                nc.tensor.matmul(q[:, 0:CH], lhsT, wt[:, k, 0:CH],
                                 start=st, stop=sp)
                nc.tensor.matmul(q[:, CH:2 * CH], lhsT, wt[:, k, CH:2 * CH],
                                 start=st, stop=sp)

            def run_pair(cp, t0, t1):
                """One pass: tiles (t0, t1) over all k for chunk-pair cp."""
                e = 2 * ((t0 // 2) % 2)
                q0 = psum.tile([P, 2 * CH], f32, tag=f"q{e}",
                               name=f"q_{cp}_{t0}")
                q1 = psum.tile([P, 2 * CH], f32, tag=f"q{e + 1}",
                               name=f"q_{cp}_{t1}")
                wt = w_sb[cp]
                for k in range(KT):
                    st, sp = (k == 0), (k == KT - 1)
                    mm(q0, t0, k, wt, st, sp)
                    mm(q1, t1, k, wt, st, sp)
                close(cp, t0, q0)
                close(cp, t1, q1)

            def run_pair_lagged(cp, t0, t1, lag=3):
                """Last pass: t1 lags so t0's close overlaps t1's tail MMs."""
                e = 2 * ((t0 // 2) % 2)
                q0 = psum.tile([P, 2 * CH], f32, tag=f"q{e}",
                               name=f"q_{cp}_{t0}")
                q1 = psum.tile([P, 2 * CH], f32, tag=f"q{e + 1}",
                               name=f"q_{cp}_{t1}")
                wt = w_sb[cp]
                for k in range(lag):
                    mm(q0, t0, k, wt, k == 0, k == KT - 1)
                for k in range(lag, KT):
                    mm(q0, t0, k, wt, k == 0, k == KT - 1)
                    mm(q1, t1, k - lag, wt, k - lag == 0, False)
                close(cp, t0, q0)
                for k in range(KT - lag, KT):
                    mm(q1, t1, k, wt, False, k == KT - 1)
                close(cp, t1, q1)

            for cp in range(NCP):
                for tp in range(TT // 2):
                    if cp == NCP - 1 and tp == TT // 2 - 1:
                        run_pair_lagged(cp, 2 * tp, 2 * tp + 1)
                    else:
                        run_pair(cp, 2 * tp, 2 * tp + 1)

    nc.compile()
    return nc


def _prep_inputs(x, W_orig, A_kernel, B_kernel):
    import ml_dtypes

    bf16 = ml_dtypes.bfloat16
    x = np.asarray(x, dtype=np.float32)
    W_orig = np.asarray(W_orig, dtype=np.float32)
    A_kernel = np.asarray(A_kernel, dtype=np.float32)
    B_kernel = np.asarray(B_kernel, dtype=np.float32)

    # Fold the LoRA update into the dense weight (exact identity):
    #   x@W + SCALE*(x@A)@B  ==  x @ (W + SCALE*A@B)
    W2 = W_orig.reshape(H, N)
    W_eff = W2 + np.float32(SCALE) * (A_kernel @ B_kernel.reshape(RANK, N))
    # [NCP, P, KT, 1024] chunk-pair major, partition-first
    w4 = np.ascontiguousarray(
        W_eff.reshape(KT, P, NCP, 2 * CH).transpose(2, 1, 0, 3).astype(bf16))

    x2d = x.reshape(TOK, H)
    in_maps = []
    for i in range(NCORES):
        xs = x2d[i * TPC:(i + 1) * TPC]                    # [TPC, H]
        # xt[p, t, k, j] = xs[t*128 + j, k*128 + p]
        xtc = np.ascontiguousarray(
            xs.reshape(TT, P, KT, P).transpose(3, 0, 2, 1).astype(bf16))
        in_maps.append({"xt": xtc, "w": w4})
    return in_maps


def kernel(x, W_orig, A_kernel, B_kernel):
    from concourse.bass_utils import run_bass_kernel_spmd

    if "nc" not in _CACHE:
        _CACHE["nc"] = _build_program()
    nc = _CACHE["nc"]

    in_maps = _prep_inputs(x, W_orig, A_kernel, B_kernel)
    res = run_bass_kernel_spmd(nc, in_maps, list(range(NCORES)))
    parts = [np.asarray(res.results[i]["out"]) for i in range(NCORES)]
    full = np.concatenate(parts, axis=0).astype(np.float32)   # [TOK, N]
    return full.reshape(B, S, NH, HD)


# revision 8
# speedup vs baseline: 1.1133x; 1.1133x over previous
"""LoRA layer kernel for Trainium2, SPMD across 8 NeuronCores.

Computes: out[b,s,h,d] = x[b,s,:] @ W_orig[:,h,d] + SCALE * (x @ A) @ B[:,h,d]

Strategy:
  - LoRA is folded on the host: W_eff = W + SCALE * (A @ B)  (exact
    algebraic identity — standard LoRA weight merge). The device kernel
    is then a pure GEMM out[8192, 2048] = x @ W_eff.
  - Data-parallel over tokens: 8192 tokens -> 1024 per core; W_eff
    replicated (8 MiB bf16 per core).
  - Per core: out[1024, 2048] = x_slice @ W_eff, accumulated over 16
    k-slabs of 128. Loop: chunk-pair (1024 output cols) outer, then
    token-tile PAIRS inner: per k-slab, 4 matmuls of N=512 share the
    256 KiB W slab, so compute (852 ns) outpaces the slab's DMA
    (~715 ns) and the PE stays busy from the first slab's arrival.
  - DMAs are batched (10 input DMAs total) to keep the Tile semaphore
    count low — the NEFF epilogue serially resets every allocated
    semaphore (~115 ns each), so sem count is directly tail latency.
  - All matmul operands bf16, PSUM accumulates fp32, output staged
    bf16 and upcast to fp32 on host.
"""

import numpy as np

# Problem shapes (hardcoded per contract - kernel.py must be self-contained)
B, S, H = 4, 2048, 2048
NH, HD = 16, 128
N = NH * HD            # 2048 output features
RANK = 4
ALPHA = 4.0
SCALE = ALPHA / RANK   # 1.0
NCORES = 8
TOK = B * S            # 8192 tokens total
TPC = TOK // NCORES    # 1024 tokens per core

P = 128                # SBUF partitions
KT = H // P            # 16 contraction slabs
TT = TPC // P          # 8 token tiles per core
CH = 512               # psum bank width (fp32)
NCP = 2                # chunk-pairs (1024 cols each)

_CACHE = {}


def _build_program():
    import concourse.mybir as mybir
    import concourse.tile as tile
    from concourse import bacc

    f32 = mybir.dt.float32
    bf16 = mybir.dt.bfloat16

    nc = bacc.Bacc(None, target_bir_lowering=False, debug=False)

    xt = nc.dram_tensor("xt", [P, TT, KT, P], bf16, kind="ExternalInput")
    w = nc.dram_tensor("w", [NCP, P, KT, 2 * CH], bf16, kind="ExternalInput")
    out = nc.dram_tensor("out", [TPC, N], bf16, kind="ExternalOutput")

    with tile.TileContext(nc) as tc:
        with (
            tc.tile_pool(name="wpool", bufs=1) as wpool,
            tc.tile_pool(name="xpool", bufs=1) as xpool,
            tc.tile_pool(name="opool", bufs=2) as opool,
            tc.tile_pool(name="psum", bufs=1, space="PSUM") as psum,
        ):
            # ---- input DMAs (sync queue, priority order, batched) ----
            # One SBUF tile per chunk-pair for W and one for x; slab-granular
            # DMA only where pacing needs it (first pass), batched after.
            w_sb = [
                wpool.tile([P, KT, 2 * CH], bf16, tag=f"w{cp}",
                           name=f"w_{cp}")
                for cp in range(NCP)
            ]
            x_sb = xpool.tile([P, TT, KT, P], bf16, tag="x", name="x_sb")

            # W streams on the sync HWDGE ring; x goes on the scalar ring in
            # parallel, so the first matmul's operands overlap in flight.
            nc.sync.dma_start(w_sb[0][:, 0:1], w[0, :, 0:1])
            nc.scalar.dma_start(x_sb[:, 0:1], xt[:, 0:1])
            nc.sync.dma_start(w_sb[0][:, 1:2], w[0, :, 1:2])
            nc.scalar.dma_start(x_sb[:, 1:2], xt[:, 1:2])
            nc.sync.dma_start(w_sb[0][:, 2:4], w[0, :, 2:4])
            nc.scalar.dma_start(x_sb[:, 2:4], xt[:, 2:4])
            nc.sync.dma_start(w_sb[0][:, 4:8], w[0, :, 4:8])
            nc.scalar.dma_start(x_sb[:, 4:TT], xt[:, 4:TT])
            nc.sync.dma_start(w_sb[0][:, 8:16], w[0, :, 8:16])
            nc.sync.dma_start(w_sb[1][:, 0:8], w[1, :, 0:8])
            nc.sync.dma_start(w_sb[1][:, 8:16], w[1, :, 8:16])

            # ---- compute: chunk-pair outer, token-tile pairs inner ----
            def close(cp, t, q):
                ot = opool.tile([P, 2 * CH], bf16, tag="o", name=f"o_{cp}_{t}")
                nc.vector.tensor_copy(ot[:], q[:])
                nc.scalar.dma_start(
                    out[t * P:(t + 1) * P, cp * 2 * CH:(cp + 1) * 2 * CH],
                    ot[:])

            def mm(q, t, k, wt, st, sp):
                lhsT = x_sb[:, t, k, :]
                nc.tensor.matmul(q[:, 0:CH], lhsT, wt[:, k, 0:CH],
                                 start=st, stop=sp)
                nc.tensor.matmul(q[:, CH:2 * CH], lhsT, wt[:, k, CH:2 * CH],
                                 start=st, stop=sp)

            def run_pair(cp, t0, t1):
                """One pass: tiles (t0, t1) over all k for chunk-pair cp."""
                e = 2 * ((t0 // 2) % 2)
                q0 = psum.tile([P, 2 * CH], f32, tag=f"q{e}",
                               name=f"q_{cp}_{t0}")
                q1 = psum.tile([P, 2 * CH], f32, tag=f"q{e + 1}",
                               name=f"q_{cp}_{t1}")
                wt = w_sb[cp]
                for k in range(KT):
                    st, sp = (k == 0), (k == KT - 1)
                    mm(q0, t0, k, wt, st, sp)
                    mm(q1, t1, k, wt, st, sp)
                close(cp, t0, q0)
                close(cp, t1, q1)

            def run_pair_lagged(cp, t0, t1, lag=3):
                """Last pass: t1 lags so t0's close overlaps t1's tail MMs."""
                e = 2 * ((t0 // 2) % 2)
                q0 = psum.tile([P, 2 * CH], f32, tag=f"q{e}",
                               name=f"q_{cp}_{t0}")
                q1 = psum.tile([P, 2 * CH], f32, tag=f"q{e + 1}",
                               name=f"q_{cp}_{t1}")
                wt = w_sb[cp]
                for k in range(lag):
                    mm(q0, t0, k, wt, k == 0, k == KT - 1)
                for k in range(lag, KT):
                    mm(q0, t0, k, wt, k == 0, k == KT - 1)
                    mm(q1, t1, k - lag, wt, k - lag == 0, False)
                close(cp, t0, q0)
                for k in range(KT - lag, KT - 1):
                    mm(q1, t1, k, wt, False, False)
                # final slab: close each psum bank as soon as its stop MM
                # retires so the copy+DMA overlap the other bank's matmul
                lhsT = x_sb[:, t1, KT - 1, :]
                ot = opool.tile([P, 2 * CH], bf16, tag="o", name=f"o_{cp}_{t1}")
                nc.tensor.matmul(q1[:, 0:CH], lhsT, wt[:, KT - 1, 0:CH],
                                 start=False, stop=True)
                nc.vector.tensor_copy(ot[:, 0:CH], q1[:, 0:CH])
                nc.scalar.dma_start(
                    out[t1 * P:(t1 + 1) * P,
                        cp * 2 * CH:cp * 2 * CH + CH],
                    ot[:, 0:CH])
                nc.tensor.matmul(q1[:, CH:2 * CH], lhsT,
                                 wt[:, KT - 1, CH:2 * CH],
                                 start=False, stop=True)
                nc.vector.tensor_copy(ot[:, CH:2 * CH], q1[:, CH:2 * CH])
                nc.scalar.dma_start(
                    out[t1 * P:(t1 + 1) * P,
                        cp * 2 * CH + CH:(cp + 1) * 2 * CH],
                    ot[:, CH:2 * CH])

            for cp in range(NCP):
                for tp in range(TT // 2):
                    if cp == NCP - 1 and tp == TT // 2 - 1:
                        run_pair_lagged(cp, 2 * tp, 2 * tp + 1)
                    else:
                        run_pair(cp, 2 * tp, 2 * tp + 1)

    nc.compile()
    return nc


def _prep_inputs(x, W_orig, A_kernel, B_kernel):
    import ml_dtypes

    bf16 = ml_dtypes.bfloat16
    x = np.asarray(x, dtype=np.float32)
    W_orig = np.asarray(W_orig, dtype=np.float32)
    A_kernel = np.asarray(A_kernel, dtype=np.float32)
    B_kernel = np.asarray(B_kernel, dtype=np.float32)

    # Fold the LoRA update into the dense weight (exact identity):
    #   x@W + SCALE*(x@A)@B  ==  x @ (W + SCALE*A@B)
    W2 = W_orig.reshape(H, N)
    W_eff = W2 + np.float32(SCALE) * (A_kernel @ B_kernel.reshape(RANK, N))
    # [NCP, P, KT, 1024] chunk-pair major, partition-first
    w4 = np.ascontiguousarray(
        W_eff.reshape(KT, P, NCP, 2 * CH).transpose(2, 1, 0, 3).astype(bf16))

    x2d = x.reshape(TOK, H)
    in_maps = []
    for i in range(NCORES):
        xs = x2d[i * TPC:(i + 1) * TPC]                    # [TPC, H]
        # xt[p, t, k, j] = xs[t*128 + j, k*128 + p]
        xtc = np.ascontiguousarray(
            xs.reshape(TT, P, KT, P).transpose(3, 0, 2, 1).astype(bf16))
        in_maps.append({"xt": xtc, "w": w4})
    return in_maps


def kernel(x, W_orig, A_kernel, B_kernel):
    from concourse.bass_utils import run_bass_kernel_spmd

    if "nc" not in _CACHE:
        _CACHE["nc"] = _build_program()
    nc = _CACHE["nc"]

    in_maps = _prep_inputs(x, W_orig, A_kernel, B_kernel)
    res = run_bass_kernel_spmd(nc, in_maps, list(range(NCORES)))
    parts = [np.asarray(res.results[i]["out"]) for i in range(NCORES)]
    full = np.concatenate(parts, axis=0).astype(np.float32)   # [TOK, N]
    return full.reshape(B, S, NH, HD)


# revision 9
# speedup vs baseline: 1.2393x; 1.1132x over previous
"""LoRA layer kernel for Trainium2, SPMD across 8 NeuronCores.

Computes: out[b,s,h,d] = x[b,s,:] @ W_orig[:,h,d] + SCALE * (x @ A) @ B[:,h,d]

Strategy:
  - LoRA is folded on the host: W_eff = W + SCALE * (A @ B)  (exact
    algebraic identity — standard LoRA weight merge). The device kernel
    is then a pure GEMM out[8192, 2048] = x @ W_eff.
  - Data-parallel over tokens: 8192 tokens -> 1024 per core; W_eff
    replicated per core.
  - Mixed precision: the first 2 of 16 k-slabs (256 of 2048
    contraction) run as ONE fp8e4m3 DoubleRow matmul per output chunk
    (2 rows/cycle), the remaining 14 slabs in bf16. Exact-sim rel err
    1.46e-2 < 2e-2 gate. This trades 2 bf16 matmuls for 1 fp8 matmul
    per (tile, chunk).
  - Loop: chunk-pair (1024 output cols) outer, then token-tile PAIRS
    inner: per k-slab, 4 matmuls of N=512 share the 256 KiB W slab, so
    compute (852 ns) outpaces the slab's DMA (~715 ns) and the PE
    stays busy from the first slab's arrival.
  - All input DMAs batched on one HWDGE ring in consumption order
    (a second ring competes for the same HBM 358 GB/s and starves the
    W stream). Output DMAs ride the scalar ring.
  - PSUM accumulates fp32, output staged bf16, upcast to fp32 on host.
"""

import numpy as np

# Problem shapes (hardcoded per contract - kernel.py must be self-contained)
B, S, H = 4, 2048, 2048
NH, HD = 16, 128
N = NH * HD            # 2048 output features
RANK = 4
ALPHA = 4.0
SCALE = ALPHA / RANK   # 1.0
NCORES = 8
TOK = B * S            # 8192 tokens total
TPC = TOK // NCORES    # 1024 tokens per core

P = 128                # SBUF partitions
KT = H // P            # 16 contraction slabs
KF8 = 2                # leading slabs done in fp8 DoubleRow
KTB = KT - KF8         # bf16 slabs
TT = TPC // P          # 8 token tiles per core
CH = 512               # psum bank width (fp32)
NCP = 2                # chunk-pairs (1024 cols each)

_CACHE = {}


def _build_program():
    import concourse.mybir as mybir
    import concourse.tile as tile
    from concourse import bacc

    f32 = mybir.dt.float32
    bf16 = mybir.dt.bfloat16
    f8 = mybir.dt.float8e4
    DR = mybir.MatmulPerfMode.DoubleRow

    nc = bacc.Bacc(None, target_bir_lowering=False, debug=False)

    x8d = nc.dram_tensor("x8", [P, TT, KF8, P], f8, kind="ExternalInput")
    w8d = nc.dram_tensor("w8", [NCP, P, KF8, 2 * CH], f8, kind="ExternalInput")
    xt = nc.dram_tensor("xt", [P, TT, KTB, P], bf16, kind="ExternalInput")
    w = nc.dram_tensor("w", [NCP, P, KTB, 2 * CH], bf16, kind="ExternalInput")
    out = nc.dram_tensor("out", [TPC, N], bf16, kind="ExternalOutput")

    with tile.TileContext(nc) as tc:
        with (
            tc.tile_pool(name="wpool", bufs=1) as wpool,
            tc.tile_pool(name="xpool", bufs=1) as xpool,
            tc.tile_pool(name="opool", bufs=2) as opool,
            tc.tile_pool(name="psum", bufs=1, space="PSUM") as psum,
        ):
            w_sb = [
                wpool.tile([P, KTB, 2 * CH], bf16, tag=f"w{cp}", name=f"w_{cp}")
                for cp in range(NCP)
            ]
            w8_sb = [
                wpool.tile([P, KF8, 2 * CH], f8, tag=f"w8{cp}", name=f"w8_{cp}")
                for cp in range(NCP)
            ]
            x_sb = xpool.tile([P, TT, KTB, P], bf16, tag="x", name="x_sb")
            x8_sb = xpool.tile([P, TT, KF8, P], f8, tag="x8", name="x8_sb")

            # ---- input DMAs: one ring, consumption order ----
            nc.sync.dma_start(x8_sb[:], x8d[:])
            nc.sync.dma_start(w8_sb[0][:], w8d[0])
            nc.sync.dma_start(x_sb[:, 0:2], xt[:, 0:2])
            nc.sync.dma_start(w_sb[0][:, 0:2], w[0, :, 0:2])
            nc.sync.dma_start(w_sb[0][:, 2:5], w[0, :, 2:5])
            nc.sync.dma_start(x_sb[:, 2:4], xt[:, 2:4])
            nc.sync.dma_start(w_sb[0][:, 5:9], w[0, :, 5:9])
            nc.sync.dma_start(x_sb[:, 4:6], xt[:, 4:6])
            nc.sync.dma_start(w_sb[0][:, 9:KTB], w[0, :, 9:KTB])
            nc.sync.dma_start(x_sb[:, 6:8], xt[:, 6:8])
            nc.sync.dma_start(w8_sb[1][:], w8d[1])
            nc.sync.dma_start(w_sb[1][:, 0:7], w[1, :, 0:7])
            nc.sync.dma_start(w_sb[1][:, 7:KTB], w[1, :, 7:KTB])

            # ---- compute: chunk-pair outer, token-tile pairs inner ----
            def close(cp, t, q):
                ot = opool.tile([P, 2 * CH], bf16, tag="o", name=f"o_{cp}_{t}")
                nc.vector.tensor_copy(ot[:], q[:])
                nc.scalar.dma_start(
                    out[t * P:(t + 1) * P, cp * 2 * CH:(cp + 1) * 2 * CH],
                    ot[:])

            def mm_dr(q, t, wt8, st):
                """fp8 DoubleRow: both leading k-slabs in one matmul/chunk."""
                a = x8_sb[:, t, 0:KF8, :]
                nc.tensor.matmul(q[:, 0:CH], a, wt8[:, 0:KF8, 0:CH],
                                 start=st, stop=False, perf_mode=DR)
                nc.tensor.matmul(q[:, CH:2 * CH], a, wt8[:, 0:KF8, CH:2 * CH],
                                 start=st, stop=False, perf_mode=DR)

            def mm(q, t, k, wt, st, sp):
                lhsT = x_sb[:, t, k, :]
                nc.tensor.matmul(q[:, 0:CH], lhsT, wt[:, k, 0:CH],
                                 start=st, stop=sp)
                nc.tensor.matmul(q[:, CH:2 * CH], lhsT, wt[:, k, CH:2 * CH],
                                 start=st, stop=sp)

            def run_pair(cp, t0, t1):
                """One pass: tiles (t0, t1) over all k for chunk-pair cp."""
                e = 2 * ((t0 // 2) % 2)
                q0 = psum.tile([P, 2 * CH], f32, tag=f"q{e}",
                               name=f"q_{cp}_{t0}")
                q1 = psum.tile([P, 2 * CH], f32, tag=f"q{e + 1}",
                               name=f"q_{cp}_{t1}")
                wt = w_sb[cp]
                mm_dr(q0, t0, w8_sb[cp], True)
                mm_dr(q1, t1, w8_sb[cp], True)
                for k in range(KTB):
                    sp = (k == KTB - 1)
                    mm(q0, t0, k, wt, False, sp)
                    mm(q1, t1, k, wt, False, sp)
                close(cp, t0, q0)
                close(cp, t1, q1)

            def run_pair_lagged(cp, t0, t1, lag=3):
                """Last pass: t1 lags so t0's close overlaps t1's tail MMs;
                t1's final slab closes per psum bank."""
                e = 2 * ((t0 // 2) % 2)
                q0 = psum.tile([P, 2 * CH], f32, tag=f"q{e}",
                               name=f"q_{cp}_{t0}")
                q1 = psum.tile([P, 2 * CH], f32, tag=f"q{e + 1}",
                               name=f"q_{cp}_{t1}")
                wt = w_sb[cp]
                mm_dr(q0, t0, w8_sb[cp], True)
                for k in range(lag - 1):
                    mm(q0, t0, k, wt, False, False)
                mm_dr(q1, t1, w8_sb[cp], True)
                for k in range(lag - 1, KTB):
                    mm(q0, t0, k, wt, False, k == KTB - 1)
                    mm(q1, t1, k - lag + 1, wt, False, False)
                close(cp, t0, q0)
                for k in range(KTB - lag + 1, KTB - 1):
                    mm(q1, t1, k, wt, False, False)
                # final slab: close each psum bank as soon as its stop MM
                # retires so the copy+DMA overlap the other bank's matmul
                lhsT = x_sb[:, t1, KTB - 1, :]
                ot = opool.tile([P, 2 * CH], bf16, tag="o", name=f"o_{cp}_{t1}")
                nc.tensor.matmul(q1[:, 0:CH], lhsT, wt[:, KTB - 1, 0:CH],
                                 start=False, stop=True)
                nc.vector.tensor_copy(ot[:, 0:CH], q1[:, 0:CH])
                nc.scalar.dma_start(
                    out[t1 * P:(t1 + 1) * P, cp * 2 * CH:cp * 2 * CH + CH],
                    ot[:, 0:CH])
                nc.tensor.matmul(q1[:, CH:2 * CH], lhsT,
                                 wt[:, KTB - 1, CH:2 * CH],
                                 start=False, stop=True)
                nc.vector.tensor_copy(ot[:, CH:2 * CH], q1[:, CH:2 * CH])
                nc.scalar.dma_start(
                    out[t1 * P:(t1 + 1) * P,
                        cp * 2 * CH + CH:(cp + 1) * 2 * CH],
                    ot[:, CH:2 * CH])

            for cp in range(NCP):
                for tp in range(TT // 2):
                    if cp == NCP - 1 and tp == TT // 2 - 1:
                        run_pair_lagged(cp, 2 * tp, 2 * tp + 1)
                    else:
                        run_pair(cp, 2 * tp, 2 * tp + 1)

    nc.compile()
    return nc


def _prep_inputs(x, W_orig, A_kernel, B_kernel):
    import ml_dtypes

    bf16 = ml_dtypes.bfloat16
    f8 = ml_dtypes.float8_e4m3
    x = np.asarray(x, dtype=np.float32)
    W_orig = np.asarray(W_orig, dtype=np.float32)
    A_kernel = np.asarray(A_kernel, dtype=np.float32)
    B_kernel = np.asarray(B_kernel, dtype=np.float32)

    # Fold the LoRA update into the dense weight (exact identity):
    #   x@W + SCALE*(x@A)@B  ==  x @ (W + SCALE*A@B)
    W2 = W_orig.reshape(H, N)
    W_eff = W2 + np.float32(SCALE) * (A_kernel @ B_kernel.reshape(RANK, N))
    KB = KF8 * P
    # fp8 slabs [NCP, P, KF8, 1024]; bf16 slabs [NCP, P, KTB, 1024]
    w8 = np.ascontiguousarray(
        W_eff[:KB].reshape(KF8, P, NCP, 2 * CH).transpose(2, 1, 0, 3)
        .astype(f8))
    w4 = np.ascontiguousarray(
        W_eff[KB:].reshape(KTB, P, NCP, 2 * CH).transpose(2, 1, 0, 3)
        .astype(bf16))

    x2d = x.reshape(TOK, H)
    in_maps = []
    for i in range(NCORES):
        xs = x2d[i * TPC:(i + 1) * TPC]                    # [TPC, H]
        # x8[p, t, ks, j] = xs[t*128 + j, ks*128 + p], ks < KF8
        x8c = np.ascontiguousarray(
            xs[:, :KB].reshape(TT, P, KF8, P).transpose(3, 0, 2, 1)
            .astype(f8))
        # xt[p, t, k, j] = xs[t*128 + j, (KF8 + k)*128 + p]
        xtc = np.ascontiguousarray(
            xs[:, KB:].reshape(TT, P, KTB, P).transpose(3, 0, 2, 1)
            .astype(bf16))
        in_maps.append({"x8": x8c, "xt": xtc, "w8": w8, "w": w4})
    return in_maps


def kernel(x, W_orig, A_kernel, B_kernel):
    from concourse.bass_utils import run_bass_kernel_spmd

    if "nc" not in _CACHE:
        _CACHE["nc"] = _build_program()
    nc = _CACHE["nc"]

    in_maps = _prep_inputs(x, W_orig, A_kernel, B_kernel)
    res = run_bass_kernel_spmd(nc, in_maps, list(range(NCORES)))
    parts = [np.asarray(res.results[i]["out"]) for i in range(NCORES)]
    full = np.concatenate(parts, axis=0).astype(np.float32)   # [TOK, N]
    return full.reshape(B, S, NH, HD)


# revision 14
# speedup vs baseline: 1.2490x; 1.0078x over previous
"""LoRA layer kernel for Trainium2, SPMD across 8 NeuronCores.

Computes: out[b,s,h,d] = x[b,s,:] @ W_orig[:,h,d] + SCALE * (x @ A) @ B[:,h,d]

Strategy:
  - LoRA is folded on the host: W_eff = W + SCALE * (A @ B)  (exact
    algebraic identity — standard LoRA weight merge). The device kernel
    is then a pure GEMM out[8192, 2048] = x @ W_eff.
  - Data-parallel over tokens: 8192 tokens -> 1024 per core; W_eff
    replicated per core.
  - Mixed precision: the first 2 of 16 k-slabs (256 of 2048
    contraction) run as ONE fp8e4m3 DoubleRow matmul per output chunk
    (2 rows/cycle), the remaining 14 slabs in bf16. Exact-sim rel err
    1.46e-2 < 2e-2 gate. This trades 2 bf16 matmuls for 1 fp8 matmul
    per (tile, chunk).
  - Loop: chunk-pair (1024 output cols) outer, then token-tile PAIRS
    inner: per k-slab, 4 matmuls of N=512 share the 256 KiB W slab, so
    compute (852 ns) outpaces the slab's DMA (~715 ns) and the PE
    stays busy from the first slab's arrival.
  - All input DMAs batched on one HWDGE ring in consumption order
    (a second ring competes for the same HBM 358 GB/s and starves the
    W stream). Output DMAs ride the scalar ring.
  - PSUM accumulates fp32, output staged bf16, upcast to fp32 on host.
"""

import numpy as np

# Problem shapes (hardcoded per contract - kernel.py must be self-contained)
B, S, H = 4, 2048, 2048
NH, HD = 16, 128
N = NH * HD            # 2048 output features
RANK = 4
ALPHA = 4.0
SCALE = ALPHA / RANK   # 1.0
NCORES = 8
TOK = B * S            # 8192 tokens total
TPC = TOK // NCORES    # 1024 tokens per core

P = 128                # SBUF partitions
KT = H // P            # 16 contraction slabs
KF8 = 2                # leading slabs done in fp8 DoubleRow
KTB = KT - KF8         # bf16 slabs
TT = TPC // P          # 8 token tiles per core
CH = 512               # psum bank width (fp32)
NCP = 2                # chunk-pairs (1024 cols each)

_CACHE = {}


def _build_program():
    import concourse.mybir as mybir
    import concourse.tile as tile
    from concourse import bacc

    f32 = mybir.dt.float32
    bf16 = mybir.dt.bfloat16
    f8 = mybir.dt.float8e4
    DR = mybir.MatmulPerfMode.DoubleRow

    nc = bacc.Bacc(None, target_bir_lowering=False, debug=False)

    x8d = nc.dram_tensor("x8", [P, TT, KF8, P], f8, kind="ExternalInput")
    w8d = nc.dram_tensor("w8", [NCP, P, KF8, 2 * CH], f8, kind="ExternalInput")
    xt = nc.dram_tensor("xt", [P, TT, KTB, P], bf16, kind="ExternalInput")
    w = nc.dram_tensor("w", [NCP, P, KTB, 2 * CH], bf16, kind="ExternalInput")
    out = nc.dram_tensor("out", [TPC, N], bf16, kind="ExternalOutput")

    with tile.TileContext(nc) as tc:
        with (
            tc.tile_pool(name="wpool", bufs=1) as wpool,
            tc.tile_pool(name="xpool", bufs=1) as xpool,
            tc.tile_pool(name="opool", bufs=2) as opool,
            tc.tile_pool(name="psum", bufs=1, space="PSUM") as psum,
        ):
            w_sb = [
                wpool.tile([P, KTB, 2 * CH], bf16, tag=f"w{cp}", name=f"w_{cp}")
                for cp in range(NCP)
            ]
            w8_sb = [
                wpool.tile([P, KF8, 2 * CH], f8, tag=f"w8{cp}", name=f"w8_{cp}")
                for cp in range(NCP)
            ]
            x_sb = xpool.tile([P, TT, KTB, P], bf16, tag="x", name="x_sb")
            x8_sb = xpool.tile([P, TT, KF8, P], f8, tag="x8", name="x8_sb")
            warm = xpool.tile([P, CH], bf16, tag="warm", name="warm")

            # ---- input DMAs: one ring, consumption order ----
            nc.sync.dma_start(x8_sb[:], x8d[:])
            nc.sync.dma_start(w8_sb[0][:], w8d[0])
            nc.sync.dma_start(x_sb[:, 0:2], xt[:, 0:2])
            nc.sync.dma_start(w_sb[0][:, 0:2], w[0, :, 0:2])
            nc.sync.dma_start(w_sb[0][:, 2:5], w[0, :, 2:5])
            nc.sync.dma_start(x_sb[:, 2:4], xt[:, 2:4])
            nc.sync.dma_start(w_sb[0][:, 5:9], w[0, :, 5:9])
            nc.sync.dma_start(x_sb[:, 4:6], xt[:, 4:6])
            nc.sync.dma_start(w_sb[0][:, 9:KTB], w[0, :, 9:KTB])
            nc.sync.dma_start(x_sb[:, 6:8], xt[:, 6:8])
            nc.sync.dma_start(w8_sb[1][:], w8d[1])
            nc.sync.dma_start(w_sb[1][:, 0:7], w[1, :, 0:7])
            nc.sync.dma_start(w_sb[1][:, 7:KTB], w[1, :, 7:KTB])

            # ---- PE prewarm: dummy matmuls while input DMAs are in flight,
            # so the HAM clock gate reaches 8/8 before the first real MM and
            # the cold 1.2 GHz window burns idle time instead of real work.
            nc.vector.memset(warm[:], 0.0)
            qw = psum.tile([P, CH], f32, tag="q0", name="q_warm")
            for _ in range(16):
                nc.tensor.matmul(qw[:], warm[:, 0:P], warm[:],
                                 start=True, stop=True)

            # ---- compute: chunk-pair outer, token-tile pairs inner ----
            def close(cp, t, q):
                ot = opool.tile([P, 2 * CH], bf16, tag="o", name=f"o_{cp}_{t}")
                nc.vector.tensor_copy(ot[:], q[:])
                nc.scalar.dma_start(
                    out[t * P:(t + 1) * P, cp * 2 * CH:(cp + 1) * 2 * CH],
                    ot[:])

            def mm_dr(q, t, wt8, st):
                """fp8 DoubleRow: both leading k-slabs in one matmul/chunk."""
                a = x8_sb[:, t, 0:KF8, :]
                nc.tensor.matmul(q[:, 0:CH], a, wt8[:, 0:KF8, 0:CH],
                                 start=st, stop=False, perf_mode=DR)
                nc.tensor.matmul(q[:, CH:2 * CH], a, wt8[:, 0:KF8, CH:2 * CH],
                                 start=st, stop=False, perf_mode=DR)

            def mm(q, t, k, wt, st, sp):
                lhsT = x_sb[:, t, k, :]
                nc.tensor.matmul(q[:, 0:CH], lhsT, wt[:, k, 0:CH],
                                 start=st, stop=sp)
                nc.tensor.matmul(q[:, CH:2 * CH], lhsT, wt[:, k, CH:2 * CH],
                                 start=st, stop=sp)

            def open_q(cp, t, tag):
                return psum.tile([P, 2 * CH], f32, tag=tag, name=f"q_{cp}_{t}")

            def run_pair(cp, t0, t1, g0, g1):
                """One pass: tiles (t0, t1) over all k for chunk-pair cp."""
                q0 = open_q(cp, t0, g0)
                q1 = open_q(cp, t1, g1)
                wt = w_sb[cp]
                mm_dr(q0, t0, w8_sb[cp], True)
                mm_dr(q1, t1, w8_sb[cp], True)
                for k in range(KTB):
                    sp = (k == KTB - 1)
                    mm(q0, t0, k, wt, False, sp)
                    mm(q1, t1, k, wt, False, sp)
                close(cp, t0, q0)
                close(cp, t1, q1)

            def run_triple(cp, ts, gs, lag=2):
                """Startup pass: 3 tiles staggered by `lag` k-slabs — 6 MMs
                per W slab keeps consumption below the DMA delivery rate, so
                the PE never outruns the W stream; staggering spreads the
                closes so the next pass's psum frees early."""
                t0, t1, t2 = ts
                q = [open_q(cp, t, g) for t, g in zip(ts, gs)]
                wt = w_sb[cp]
                mm_dr(q[0], t0, w8_sb[cp], True)
                for k in range(lag):
                    mm(q[0], t0, k, wt, False, False)
                mm_dr(q[1], t1, w8_sb[cp], True)
                for k in range(lag, 2 * lag):
                    mm(q[0], t0, k, wt, False, False)
                    mm(q[1], t1, k - lag, wt, False, False)
                mm_dr(q[2], t2, w8_sb[cp], True)
                for k in range(2 * lag, KTB):
                    mm(q[0], t0, k, wt, False, k == KTB - 1)
                    mm(q[1], t1, k - lag, wt, False, False)
                    mm(q[2], t2, k - 2 * lag, wt, False, False)
                close(cp, t0, q[0])
                for k in range(KTB - lag, KTB):
                    mm(q[1], t1, k, wt, False, k == KTB - 1)
                    mm(q[2], t2, k - lag, wt, False, False)
                close(cp, t1, q[1])
                for k in range(KTB - lag, KTB):
                    mm(q[2], t2, k, wt, False, k == KTB - 1)
                close(cp, t2, q[2])

            def run_single(cp, t, g):
                q = open_q(cp, t, g)
                wt = w_sb[cp]
                mm_dr(q, t, w8_sb[cp], True)
                for k in range(KTB):
                    mm(q, t, k, wt, False, k == KTB - 1)
                close(cp, t, q)

            def run_pair_lagged(cp, t0, t1, g0, g1, lag=3):
                """Last pass: t1 lags so t0's close overlaps t1's tail MMs;
                t1's final slab closes per psum bank."""
                q0 = open_q(cp, t0, g0)
                q1 = open_q(cp, t1, g1)
                wt = w_sb[cp]
                mm_dr(q0, t0, w8_sb[cp], True)
                for k in range(lag - 1):
                    mm(q0, t0, k, wt, False, False)
                mm_dr(q1, t1, w8_sb[cp], True)
                for k in range(lag - 1, KTB):
                    mm(q0, t0, k, wt, False, k == KTB - 1)
                    mm(q1, t1, k - lag + 1, wt, False, False)
                close(cp, t0, q0)
                for k in range(KTB - lag + 1, KTB - 1):
                    mm(q1, t1, k, wt, False, False)
                # final slab: close each psum bank as soon as its stop MM
                # retires so the copy+DMA overlap the other bank's matmul
                lhsT = x_sb[:, t1, KTB - 1, :]
                ot = opool.tile([P, 2 * CH], bf16, tag="o", name=f"o_{cp}_{t1}")
                nc.tensor.matmul(q1[:, 0:CH], lhsT, wt[:, KTB - 1, 0:CH],
                                 start=False, stop=True)
                nc.vector.tensor_copy(ot[:, 0:CH], q1[:, 0:CH])
                nc.scalar.dma_start(
                    out[t1 * P:(t1 + 1) * P, cp * 2 * CH:cp * 2 * CH + CH],
                    ot[:, 0:CH])
                nc.tensor.matmul(q1[:, CH:2 * CH], lhsT,
                                 wt[:, KTB - 1, CH:2 * CH],
                                 start=False, stop=True)
                nc.vector.tensor_copy(ot[:, CH:2 * CH], q1[:, CH:2 * CH])
                nc.scalar.dma_start(
                    out[t1 * P:(t1 + 1) * P,
                        cp * 2 * CH + CH:(cp + 1) * 2 * CH],
                    ot[:, CH:2 * CH])

            run_triple(0, (0, 1, 2), ("q0", "q1", "q2"))
            run_pair(0, 3, 4, "q3", "q0")
            run_pair(0, 5, 6, "q1", "q2")
            run_single(0, 7, "q3")
            run_pair(1, 0, 1, "q0", "q1")
            run_pair(1, 2, 3, "q2", "q3")
            run_pair(1, 4, 5, "q0", "q1")
            run_pair_lagged(1, 6, 7, "q2", "q3")

    nc.compile()
    return nc


def _prep_inputs(x, W_orig, A_kernel, B_kernel):
    import ml_dtypes

    bf16 = ml_dtypes.bfloat16
    f8 = ml_dtypes.float8_e4m3
    x = np.asarray(x, dtype=np.float32)
    W_orig = np.asarray(W_orig, dtype=np.float32)
    A_kernel = np.asarray(A_kernel, dtype=np.float32)
    B_kernel = np.asarray(B_kernel, dtype=np.float32)

    # Fold the LoRA update into the dense weight (exact identity):
    #   x@W + SCALE*(x@A)@B  ==  x @ (W + SCALE*A@B)
    W2 = W_orig.reshape(H, N)
    W_eff = W2 + np.float32(SCALE) * (A_kernel @ B_kernel.reshape(RANK, N))
    KB = KF8 * P
    # fp8 slabs [NCP, P, KF8, 1024]; bf16 slabs [NCP, P, KTB, 1024]
    w8 = np.ascontiguousarray(
        W_eff[:KB].reshape(KF8, P, NCP, 2 * CH).transpose(2, 1, 0, 3)
        .astype(f8))
    w4 = np.ascontiguousarray(
        W_eff[KB:].reshape(KTB, P, NCP, 2 * CH).transpose(2, 1, 0, 3)
        .astype(bf16))

    x2d = x.reshape(TOK, H)
    in_maps = []
    for i in range(NCORES):
        xs = x2d[i * TPC:(i + 1) * TPC]                    # [TPC, H]
        # x8[p, t, ks, j] = xs[t*128 + j, ks*128 + p], ks < KF8
        x8c = np.ascontiguousarray(
            xs[:, :KB].reshape(TT, P, KF8, P).transpose(3, 0, 2, 1)
            .astype(f8))
        # xt[p, t, k, j] = xs[t*128 + j, (KF8 + k)*128 + p]
        xtc = np.ascontiguousarray(
            xs[:, KB:].reshape(TT, P, KTB, P).transpose(3, 0, 2, 1)
            .astype(bf16))
        in_maps.append({"x8": x8c, "xt": xtc, "w8": w8, "w": w4})
    return in_maps


def kernel(x, W_orig, A_kernel, B_kernel):
    from concourse.bass_utils import run_bass_kernel_spmd

    if "nc" not in _CACHE:
        _CACHE["nc"] = _build_program()
    nc = _CACHE["nc"]

    in_maps = _prep_inputs(x, W_orig, A_kernel, B_kernel)
    res = run_bass_kernel_spmd(nc, in_maps, list(range(NCORES)))
    parts = [np.asarray(res.results[i]["out"]) for i in range(NCORES)]
    full = np.concatenate(parts, axis=0).astype(np.float32)   # [TOK, N]
    return full.reshape(B, S, NH, HD)


# revision 15
# speedup vs baseline: 1.2521x; 1.0024x over previous
"""LoRA layer kernel for Trainium2, SPMD across 8 NeuronCores.

Computes: out[b,s,h,d] = x[b,s,:] @ W_orig[:,h,d] + SCALE * (x @ A) @ B[:,h,d]

Strategy:
  - LoRA is folded on the host: W_eff = W + SCALE * (A @ B)  (exact
    algebraic identity — standard LoRA weight merge). The device kernel
    is then a pure GEMM out[8192, 2048] = x @ W_eff.
  - Data-parallel over tokens: 8192 tokens -> 1024 per core; W_eff
    replicated per core.
  - Mixed precision: 2 of 16 k-slabs (256 of 2048 contraction) run as
    one fp8e4m3 DoubleRow matmul per output chunk (2 rows/cycle), the
    rest in bf16. Exact-sim rel err 1.46e-2 < 2e-2 gate. The DR
    matmuls close each accumulation group (not open it): mid-stream
    their 256-col LDWEIGHTS hides under the preceding matmul; at a
    pass boundary a semaphore wait would block the pull-ahead.
  - Loop: chunk-pair (1024 output cols) outer, token tiles inner
    (triple first pass, then pairs): >=4 N=512 matmuls share each
    256 KiB W slab so compute outpaces the W stream.
  - All input DMAs on one HWDGE ring in consumption order (a second
    ring competes for HBM and starves the W stream); early slabs ship
    as single-slab DMAs because the completion SEMAPHORE lags the data
    by ~1.5-2 us — small first transfers gate compute sooner.
  - PE prewarm: dummy matmuls during the DMA-fill window lift the HAM
    clock gate to 8/8 before real work arrives.
  - PSUM accumulates fp32, output staged bf16, upcast to fp32 on host.
"""

import numpy as np

# Problem shapes (hardcoded per contract - kernel.py must be self-contained)
B, S, H = 4, 2048, 2048
NH, HD = 16, 128
N = NH * HD            # 2048 output features
RANK = 4
ALPHA = 4.0
SCALE = ALPHA / RANK   # 1.0
NCORES = 8
TOK = B * S            # 8192 tokens total
TPC = TOK // NCORES    # 1024 tokens per core

P = 128                # SBUF partitions
KT = H // P            # 16 contraction slabs
KF8 = 2                # trailing slabs done in fp8 DoubleRow
KTB = KT - KF8         # bf16 slabs
TT = TPC // P          # 8 token tiles per core
CH = 512               # psum bank width (fp32)
NCP = 2                # chunk-pairs (1024 cols each)

_CACHE = {}


def _build_program():
    import concourse.mybir as mybir
    import concourse.tile as tile
    from concourse import bacc

    f32 = mybir.dt.float32
    bf16 = mybir.dt.bfloat16
    f8 = mybir.dt.float8e4
    DR = mybir.MatmulPerfMode.DoubleRow

    nc = bacc.Bacc(None, target_bir_lowering=False, debug=False)

    x8d = nc.dram_tensor("x8", [P, TT, KF8, P], f8, kind="ExternalInput")
    w8d = nc.dram_tensor("w8", [NCP, P, KF8, 2 * CH], f8, kind="ExternalInput")
    xt = nc.dram_tensor("xt", [P, TT, KTB, P], bf16, kind="ExternalInput")
    w = nc.dram_tensor("w", [NCP, P, KTB, 2 * CH], bf16, kind="ExternalInput")
    out = nc.dram_tensor("out", [TPC, N], bf16, kind="ExternalOutput")

    with tile.TileContext(nc) as tc:
        with (
            tc.tile_pool(name="wpool", bufs=1) as wpool,
            tc.tile_pool(name="xpool", bufs=1) as xpool,
            tc.tile_pool(name="opool", bufs=2) as opool,
            tc.tile_pool(name="psum", bufs=1, space="PSUM") as psum,
        ):
            w_sb = [
                wpool.tile([P, KTB, 2 * CH], bf16, tag=f"w{cp}", name=f"w_{cp}")
                for cp in range(NCP)
            ]
            w8_sb = [
                wpool.tile([P, KF8, 2 * CH], f8, tag=f"w8{cp}", name=f"w8_{cp}")
                for cp in range(NCP)
            ]
            x_sb = xpool.tile([P, TT, KTB, P], bf16, tag="x", name="x_sb")
            x8_sb = xpool.tile([P, TT, KF8, P], f8, tag="x8", name="x8_sb")
            warm = xpool.tile([P, CH], bf16, tag="warm", name="warm")

            # ---- input DMAs: one ring, consumption order; early W slabs as
            # singles so their completion sems land before compute needs them
            nc.sync.dma_start(x_sb[:, 0:1], xt[:, 0:1])
            nc.sync.dma_start(w_sb[0][:, 0:1], w[0, :, 0:1])
            nc.sync.dma_start(x_sb[:, 1:2], xt[:, 1:2])
            nc.sync.dma_start(w_sb[0][:, 1:2], w[0, :, 1:2])
            nc.sync.dma_start(w_sb[0][:, 2:3], w[0, :, 2:3])
            nc.sync.dma_start(x_sb[:, 2:3], xt[:, 2:3])
            nc.sync.dma_start(w_sb[0][:, 3:4], w[0, :, 3:4])
            nc.sync.dma_start(w_sb[0][:, 4:5], w[0, :, 4:5])
            nc.sync.dma_start(x_sb[:, 3:4], xt[:, 3:4])
            nc.sync.dma_start(w_sb[0][:, 5:7], w[0, :, 5:7])
            nc.sync.dma_start(x8_sb[:], x8d[:])
            nc.sync.dma_start(w8_sb[0][:], w8d[0])
            nc.sync.dma_start(x_sb[:, 4:6], xt[:, 4:6])
            nc.sync.dma_start(w_sb[0][:, 7:9], w[0, :, 7:9])
            nc.sync.dma_start(w_sb[0][:, 9:11], w[0, :, 9:11])
            nc.sync.dma_start(x_sb[:, 6:8], xt[:, 6:8])
            nc.sync.dma_start(w_sb[0][:, 11:KTB], w[0, :, 11:KTB])
            nc.sync.dma_start(w8_sb[1][:], w8d[1])
            nc.sync.dma_start(w_sb[1][:, 0:7], w[1, :, 0:7])
            nc.sync.dma_start(w_sb[1][:, 7:KTB], w[1, :, 7:KTB])

            # ---- PE prewarm: dummy matmuls while input DMAs are in flight,
            # so the HAM clock gate reaches 8/8 before the first real MM and
            # the cold 1.2 GHz window burns idle time instead of real work.
            nc.vector.memset(warm[:], 0.0)
            qw = psum.tile([P, CH], f32, tag="q0", name="q_warm")
            for _ in range(16):
                nc.tensor.matmul(qw[:], warm[:, 0:P], warm[:],
                                 start=True, stop=True)

            # ---- compute: chunk-pair outer, token tiles inner ----
            def close(cp, t, q):
                ot = opool.tile([P, 2 * CH], bf16, tag="o", name=f"o_{cp}_{t}")
                nc.vector.tensor_copy(ot[:], q[:])
                nc.scalar.dma_start(
                    out[t * P:(t + 1) * P, cp * 2 * CH:(cp + 1) * 2 * CH],
                    ot[:])

            def mm_dr(q, t, wt8):
                """fp8 DoubleRow: both trailing k-slabs in one matmul/chunk;
                closes the accumulation group (stop=True)."""
                a = x8_sb[:, t, 0:KF8, :]
                nc.tensor.matmul(q[:, 0:CH], a, wt8[:, 0:KF8, 0:CH],
                                 start=False, stop=True, perf_mode=DR)
                nc.tensor.matmul(q[:, CH:2 * CH], a, wt8[:, 0:KF8, CH:2 * CH],
                                 start=False, stop=True, perf_mode=DR)

            def mm(q, t, k, wt, st):
                lhsT = x_sb[:, t, k, :]
                nc.tensor.matmul(q[:, 0:CH], lhsT, wt[:, k, 0:CH],
                                 start=st, stop=False)
                nc.tensor.matmul(q[:, CH:2 * CH], lhsT, wt[:, k, CH:2 * CH],
                                 start=st, stop=False)

            def open_q(cp, t, tag):
                return psum.tile([P, 2 * CH], f32, tag=tag, name=f"q_{cp}_{t}")

            def finish(cp, t, q):
                mm_dr(q, t, w8_sb[cp])
                close(cp, t, q)

            def run_pair(cp, t0, t1, g0, g1):
                """One pass: tiles (t0, t1) over all k for chunk-pair cp."""
                q0 = open_q(cp, t0, g0)
                q1 = open_q(cp, t1, g1)
                wt = w_sb[cp]
                for k in range(KTB):
                    mm(q0, t0, k, wt, k == 0)
                    mm(q1, t1, k, wt, k == 0)
                finish(cp, t0, q0)
                finish(cp, t1, q1)

            def run_triple(cp, ts, gs, lag=2):
                """Startup pass: 3 tiles staggered by `lag` k-slabs — 6 MMs
                per W slab keeps consumption below the DMA delivery rate, so
                the PE never outruns the W stream; staggering spreads the
                closes so the next pass's psum frees early."""
                t0, t1, t2 = ts
                q = [open_q(cp, t, g) for t, g in zip(ts, gs)]
                wt = w_sb[cp]
                for k in range(lag):
                    mm(q[0], t0, k, wt, k == 0)
                for k in range(lag, 2 * lag):
                    mm(q[0], t0, k, wt, False)
                    mm(q[1], t1, k - lag, wt, k - lag == 0)
                for k in range(2 * lag, KTB):
                    mm(q[0], t0, k, wt, False)
                    mm(q[1], t1, k - lag, wt, False)
                    mm(q[2], t2, k - 2 * lag, wt, k - 2 * lag == 0)
                finish(cp, t0, q[0])
                for k in range(KTB - lag, KTB):
                    mm(q[1], t1, k, wt, False)
                    mm(q[2], t2, k - lag, wt, False)
                finish(cp, t1, q[1])
                for k in range(KTB - lag, KTB):
                    mm(q[2], t2, k, wt, False)
                finish(cp, t2, q[2])

            def run_single(cp, t, g):
                q = open_q(cp, t, g)
                wt = w_sb[cp]
                for k in range(KTB):
                    mm(q, t, k, wt, k == 0)
                finish(cp, t, q)

            def run_pair_lagged(cp, t0, t1, g0, g1, lag=3):
                """Last pass: t1 lags so t0's close overlaps t1's tail MMs;
                t1 closes each psum bank right after its DR stop."""
                q0 = open_q(cp, t0, g0)
                q1 = open_q(cp, t1, g1)
                wt = w_sb[cp]
                for k in range(lag):
                    mm(q0, t0, k, wt, k == 0)
                for k in range(lag, KTB):
                    mm(q0, t0, k, wt, False)
                    mm(q1, t1, k - lag, wt, k - lag == 0)
                finish(cp, t0, q0)
                for k in range(KTB - lag, KTB):
                    mm(q1, t1, k, wt, False)
                # per-bank close: copy+DMA of bank A overlap bank B's DR MM
                a = x8_sb[:, t1, 0:KF8, :]
                wt8 = w8_sb[cp]
                ot = opool.tile([P, 2 * CH], bf16, tag="o", name=f"o_{cp}_{t1}")
                nc.tensor.matmul(q1[:, 0:CH], a, wt8[:, 0:KF8, 0:CH],
                                 start=False, stop=True, perf_mode=DR)
                nc.vector.tensor_copy(ot[:, 0:CH], q1[:, 0:CH])
                nc.scalar.dma_start(
                    out[t1 * P:(t1 + 1) * P, cp * 2 * CH:cp * 2 * CH + CH],
                    ot[:, 0:CH])
                nc.tensor.matmul(q1[:, CH:2 * CH], a,
                                 wt8[:, 0:KF8, CH:2 * CH],
                                 start=False, stop=True, perf_mode=DR)
                nc.vector.tensor_copy(ot[:, CH:2 * CH], q1[:, CH:2 * CH])
                nc.scalar.dma_start(
                    out[t1 * P:(t1 + 1) * P,
                        cp * 2 * CH + CH:(cp + 1) * 2 * CH],
                    ot[:, CH:2 * CH])

            run_triple(0, (0, 1, 2), ("q0", "q1", "q2"))
            run_pair(0, 3, 4, "q3", "q0")
            run_pair(0, 5, 6, "q1", "q2")
            run_single(0, 7, "q3")
            run_pair(1, 0, 1, "q0", "q1")
            run_pair(1, 2, 3, "q2", "q3")
            run_pair(1, 4, 5, "q0", "q1")
            run_pair_lagged(1, 6, 7, "q2", "q3")

    nc.compile()
    return nc


def _prep_inputs(x, W_orig, A_kernel, B_kernel):
    import ml_dtypes

    bf16 = ml_dtypes.bfloat16
    f8 = ml_dtypes.float8_e4m3
    x = np.asarray(x, dtype=np.float32)
    W_orig = np.asarray(W_orig, dtype=np.float32)
    A_kernel = np.asarray(A_kernel, dtype=np.float32)
    B_kernel = np.asarray(B_kernel, dtype=np.float32)

    # Fold the LoRA update into the dense weight (exact identity):
    #   x@W + SCALE*(x@A)@B  ==  x @ (W + SCALE*A@B)
    W2 = W_orig.reshape(H, N)
    W_eff = W2 + np.float32(SCALE) * (A_kernel @ B_kernel.reshape(RANK, N))
    KB = KF8 * P
    # fp8 slabs are the LAST KF8 k-slabs; bf16 slabs the first KTB
    w8 = np.ascontiguousarray(
        W_eff[H - KB:].reshape(KF8, P, NCP, 2 * CH).transpose(2, 1, 0, 3)
        .astype(f8))
    w4 = np.ascontiguousarray(
        W_eff[:H - KB].reshape(KTB, P, NCP, 2 * CH).transpose(2, 1, 0, 3)
        .astype(bf16))

    x2d = x.reshape(TOK, H)
    in_maps = []
    for i in range(NCORES):
        xs = x2d[i * TPC:(i + 1) * TPC]                    # [TPC, H]
        # x8[p, t, ks, j] = xs[t*128 + j, (KTB + ks)*128 + p]
        x8c = np.ascontiguousarray(
            xs[:, H - KB:].reshape(TT, P, KF8, P).transpose(3, 0, 2, 1)
            .astype(f8))
        # xt[p, t, k, j] = xs[t*128 + j, k*128 + p], k < KTB
        xtc = np.ascontiguousarray(
            xs[:, :H - KB].reshape(TT, P, KTB, P).transpose(3, 0, 2, 1)
            .astype(bf16))
        in_maps.append({"x8": x8c, "xt": xtc, "w8": w8, "w": w4})
    return in_maps


def kernel(x, W_orig, A_kernel, B_kernel):
    from concourse.bass_utils import run_bass_kernel_spmd

    if "nc" not in _CACHE:
        _CACHE["nc"] = _build_program()
    nc = _CACHE["nc"]

    in_maps = _prep_inputs(x, W_orig, A_kernel, B_kernel)
    res = run_bass_kernel_spmd(nc, in_maps, list(range(NCORES)))
    parts = [np.asarray(res.results[i]["out"]) for i in range(NCORES)]
    full = np.concatenate(parts, axis=0).astype(np.float32)   # [TOK, N]
    return full.reshape(B, S, NH, HD)
